# revision 4
# baseline (speedup 1.0000x reference)
"""Trainium2 Bass kernel v2 for ResNet BasicBlock (conv3x3-BN-conv3x3-+x-BN).

Data-parallel over 8 cores (4 images each). Per core:

- Conv as tap-packed matmuls: each image is stored as 128-partition "panels"
  [plain | shifted] so a single K=128 matmul covers TWO conv taps at once.
  conv1: 5 matmuls / image / spatial tile (3 row-pairs + 1 col-pair + 1
  single), conv2: 6 (3 row-pairs + 3 singles). Two images run per tile-step
  on the two PE column halves (tile_position).
- BN1 is folded into conv2: scale into w2 (w2s = w2 * s1 per input channel),
  bias via a constant C[o] = sum_i,tap w2[o,i,tap]*b1[i] subtracted at
  eviction plus small border fixups (conv padding makes the bias field
  non-constant only at the image border).
- conv2 writes (psum - C + x_residual) in place into the panels' plain
  halves; BN2 stats accumulate via fused accum_out on the eviction ops;
  batch stats all-reduced across cores ([64,2] f32 AllReduce per BN).
- Phase C applies BN2 affine (ACT half / DVE half) and streams out f32.
"""

import os
import sys

sys.path.insert(0, "/opt/trn_rl_repo")

import numpy as np
import ml_dtypes

from contextlib import ExitStack

from concourse import bacc, bass, mybir, tile
from concourse.bass_utils import run_bass_kernel_spmd

F32 = mybir.dt.float32
BF16 = mybir.dt.bfloat16
ADD = mybir.AluOpType.add
MULT = mybir.AluOpType.mult
SUB = mybir.AluOpType.subtract
AF = mybir.ActivationFunctionType
AX = mybir.AxisListType.X
AXY = mybir.AxisListType.XY

N_CORES = 8
N_IMG = 32
C = 64
H = W = 128
HP = WP = 130
RT = 4            # rows per spatial tile
NT = H // RT      # 32 tiles per image pair
CHT = 4           # tiles per x chunk
NCH = NT // CHT   # 8 chunks
CHR = CHT * RT + 2  # 18 padded rows per P chunk window
SCH = 16          # rows per xres / phase-C / sumsq2 chunk
NHW = N_IMG * H * W
EPS = 1e-5


def _build_bass(n_cores=N_CORES, nhw=NHW, phase=""):
    nc = bacc.Bacc(
        "TRN2", target_bir_lowering=False, debug=False, num_devices=n_cores
    )

    xP = nc.dram_tensor("xP", [4, 128, HP, WP], BF16, kind="ExternalInput")
    xQ = nc.dram_tensor("xQ", [4, 128, HP, WP], BF16, kind="ExternalInput")
    xR = nc.dram_tensor("xR", [2, 128, H, W], BF16, kind="ExternalInput")
    w1Pd = nc.dram_tensor("w1Pd", [128, 3, C], BF16, kind="ExternalInput")
    w1Qd = nc.dram_tensor("w1Qd", [128, C], BF16, kind="ExternalInput")
    w1Sd = nc.dram_tensor("w1Sd", [C, C], BF16, kind="ExternalInput")
    w2Ad = nc.dram_tensor("w2Ad", [128, 3, C], BF16, kind="ExternalInput")
    w2Bd = nc.dram_tensor("w2Bd", [128, 3, C], BF16, kind="ExternalInput")
    w2Sd = nc.dram_tensor("w2Sd", [128, 3, C], BF16, kind="ExternalInput")
    w2Td = nc.dram_tensor("w2Td", [C, 9, C], BF16, kind="ExternalInput")
    w2Qd = nc.dram_tensor("w2Qd", [128, C], BF16, kind="ExternalInput")
    prmd = nc.dram_tensor("prmd", [128, 4], F32, kind="ExternalInput")
    out = nc.dram_tensor("out", [4, C, H, W], F32, kind="ExternalOutput")

    rg8 = [list(range(n_cores))]

    def _emit(tc, ctx):
        const = ctx.enter_context(tc.tile_pool(name="const", bufs=1))
        panels = ctx.enter_context(tc.tile_pool(name="panels", bufs=1))
        xpp = ctx.enter_context(tc.tile_pool(name="xpp", bufs=4))
        xqp = ctx.enter_context(tc.tile_pool(name="xqp", bufs=3))
        xrp = ctx.enter_context(tc.tile_pool(name="xrp", bufs=4))
        sqp = ctx.enter_context(tc.tile_pool(name="sqp", bufs=2))
        stat = ctx.enter_context(tc.tile_pool(name="stat", bufs=1))
        stg = ctx.enter_context(tc.tile_pool(name="stg", bufs=4))
        qp = ctx.enter_context(tc.tile_pool(name="qp", bufs=3))
        psum = ctx.enter_context(tc.tile_pool(name="psum", bufs=4, space="PSUM"))
        psb = ctx.enter_context(tc.tile_pool(name="psb", bufs=1, space="PSUM"))
        dram = ctx.enter_context(tc.tile_pool(name="dram", bufs=4, space="DRAM"))

        # ---- weights / params ----
        w1P = const.tile([128, 3, C], BF16, tag="w1P")
        w1Q = const.tile([128, C], BF16, tag="w1Q")
        w1S = const.tile([C, C], BF16, tag="w1S")
        w2A = const.tile([128, 3, C], BF16, tag="w2A")
        w2B = const.tile([128, 3, C], BF16, tag="w2B")
        w2S = const.tile([128, 3, C], BF16, tag="w2S")
        w2T = const.tile([C, 9, C], BF16, tag="w2T")
        w2Q = const.tile([128, C], BF16, tag="w2Q")
        prm = const.tile([128, 4], F32, tag="prm")
        for sb, d in ((w1P, w1Pd), (w1Q, w1Qd), (w1S, w1Sd), (w2A, w2Ad),
                      (w2B, w2Bd), (w2S, w2Sd), (w2T, w2Td), (w2Q, w2Qd),
                      (prm, prmd)):
            nc.scalar.dma_start(out=sb[:], in_=d[:])
        ones = const.tile([C, 1, W], BF16, tag="ones")
        nc.gpsimd.memset(ones[:], 1.0)

        # scaled conv2 weights (built after cc1)
        w2sA = const.tile([128, 3, C], BF16, tag="w2sA")
        w2sB = const.tile([128, 3, C], BF16, tag="w2sB")
        w2sS = const.tile([128, 3, C], BF16, tag="w2sS")
        w2sQ = const.tile([128, C], BF16, tag="w2sQ")

        # ---- persistent panels (conv1 out -> conv2 in -> o3) ----
        # pan[img]: [plain-img | shift(1,0)-img] for even img (A-layout),
        #           [shift(1,0)-img | plain-img] for odd img (B-layout).
        pan = [
            panels.tile([128, HP, WP], BF16, tag=f"pan{i}", name=f"pan{i}")
            for i in range(4)
        ]
        for p in pan:
            nc.gpsimd.memset(p[:, 0:1, :], 0.0)
            nc.gpsimd.memset(p[:, HP - 1 : HP, :], 0.0)
            nc.gpsimd.memset(p[:, :, 0:1], 0.0)
            nc.gpsimd.memset(p[:, :, WP - 1 : WP], 0.0)

        # ---- stats ----
        st1 = stat.tile([128, 2 * NT], F32, tag="st1")
        st1q = stat.tile([128, 2 * NT], F32, tag="st1q")
        st2 = stat.tile([128, 2 * NT], F32, tag="st2")
        st2q = stat.tile([128, 2 * H // SCH], F32, tag="st2q")
        red1 = stat.tile([128, 2], F32, tag="red1")
        red2 = stat.tile([128, 2], F32, tag="red2")

        # bias-field / coeff tiles
        fixT = stat.tile([128, 1, W], F32, tag="fixT")
        fixB = stat.tile([128, 1, W], F32, tag="fixB")
        sc = stat.tile([128, 8], F32, tag="sc")
        # sc cols: 0=C, 1=colL, 2=colR, 3=corr2, 4=s2, 5=b2
        m1 = stat.tile([128, 8], F32, tag="m1")
        m2 = stat.tile([128, 8], F32, tag="m2")
        tv = stat.tile([C, 3, 3], F32, tag="tv")
        s1f = stat.tile([128, 1], F32, tag="s1f")
        b1c = stat.tile([C, 1], BF16, tag="b1c")

        def bn_coeffs(tot, gcol, bcol, m, s_out, b_out):
            """tot [128,2] (sum, sumsq) -> scale/bias [128,1] f32 (dual-half)."""
            nc.vector.tensor_scalar(m[:, 0:2], tot[:, 0:2], 1.0 / nhw, None, MULT)
            nc.vector.tensor_tensor(m[:, 2:3], m[:, 0:1], m[:, 0:1], MULT)
            nc.vector.tensor_scalar(
                m[:, 3:4], m[:, 1:2], m[:, 2:3], EPS, SUB, op1=ADD
            )  # var + eps
            nc.vector.reciprocal(m[:, 6:7], m[:, 3:4])
            nc.scalar.activation(m[:, 4:5], m[:, 6:7], AF.Sqrt)
            nc.vector.tensor_tensor(
                s_out, prm[:, gcol : gcol + 1], m[:, 4:5], MULT
            )
            nc.vector.tensor_tensor(m[:, 5:6], m[:, 0:1], s_out, MULT)
            nc.vector.tensor_tensor(
                b_out, prm[:, bcol : bcol + 1], m[:, 5:6], SUB
            )

        def do_collective(src, cc_name):
            # AllGather the raw per-core [128,2] (sum, sumsq) stats, then
            # reduce over (core, partition-half) on-chip. An AllGather of
            # this size is ~2x cheaper than an AllReduce and needs no
            # pre-fold DMAs.
            cc_in = dram.tile([128, 2], F32, tag=cc_name + "i")
            cc_out = dram.tile([n_cores, 2, C, 2], F32, tag=cc_name + "o")
            nc.sync.dma_start(out=cc_in[:], in_=src[:])
            if os.environ.get("KERNEL_NOCC"):
                nc.sync.dma_start(out=cc_out[0, 0], in_=cc_in[0:C, :])
                nc.sync.dma_start(out=cc_out[0, 1], in_=cc_in[C:128, :])
                for k in range(1, n_cores):
                    nc.gpsimd.memset(cc_out[k], 0.0)
            else:
                nc.gpsimd.collective_compute(
                    "AllGather", mybir.AluOpType.bypass, replica_groups=rg8,
                    ins=[cc_in[:].opt()], outs=[cc_out[:].opt()],
                )
            g = stat.tile([128, 2, 2 * n_cores], F32, tag=cc_name + "g")
            gv = cc_out[:].rearrange("k h c s -> c s (k h)")
            nc.sync.dma_start(out=g[0:C], in_=gv)
            nc.scalar.dma_start(out=g[C:128], in_=gv)
            tot = stat.tile([128, 2], F32, tag=cc_name + "t")
            nc.vector.tensor_reduce(tot[:], g[:], AX, ADD)
            return tot

        # ================= Phase A: conv1 =================
        for pair in (0, 1):
            pA, pB = pan[2 * pair], pan[2 * pair + 1]
            for ch in range(NCH):
                r0 = CHT * RT * ch
                cps, cqs = [], []
                for i01 in (0, 1):
                    img = 2 * pair + i01
                    cp = xpp.tile([128, CHR, WP], BF16, tag="xp")
                    nc.sync.dma_start(out=cp[:], in_=xP[img][:, r0 : r0 + CHR, :])
                    cq = xqp.tile([128, CHR - 2, WP], BF16, tag="xq")
                    nc.sync.dma_start(
                        out=cq[:], in_=xQ[img][:, r0 + 2 : r0 + CHR, :]
                    )
                    cps.append(cp)
                    cqs.append(cq)
                for tl in range(CHT):
                    t = CHT * ch + tl
                    si = NT * pair + t
                    l0 = RT * tl
                    ps = psum.tile([128, RT, W], F32, tag="ps")
                    for i01 in (0, 1):
                        tp = (0, 64 * i01)
                        po = ps[64 * i01 : 64 * i01 + 64, :, :]
                        cp, cq = cps[i01], cqs[i01]
                        for kx in range(3):
                            nc.tensor.matmul(
                                po, w1P[:, kx, :],
                                cp[:, l0 : l0 + RT, kx : kx + W],
                                start=(kx == 0), stop=False, tile_position=tp,
                            )
                        nc.tensor.matmul(
                            po, w1Q[:, :], cq[:, l0 : l0 + RT, 0:W],
                            start=False, stop=False, tile_position=tp,
                        )
                        nc.tensor.matmul(
                            po, w1S[:, :],
                            cp[0:C, l0 + 2 : l0 + 2 + RT, 2 : 2 + W],
                            start=False, stop=True, tile_position=tp,
                        )
                    # evictions + stats (DVE copies, ACT squares from psum)
                    nc.vector.tensor_scalar(
                        pA[0:C, 1 + RT * t : 1 + RT * t + RT, 1 : 1 + W],
                        ps[0:C], 1.0, 0.0, MULT, op1=ADD,
                        accum_out=st1[0:C, si : si + 1],
                    )
                    nc.vector.tensor_scalar(
                        pB[C:128, 1 + RT * t : 1 + RT * t + RT, 1 : 1 + W],
                        ps[C:128], 1.0, 0.0, MULT, op1=ADD,
                        accum_out=st1[C:128, si : si + 1],
                    )
                    sq = sqp.tile([128, RT, W], BF16, tag="sq")
                    nc.scalar.activation(
                        sq[0:C], ps[0:C], AF.Square,
                        accum_out=st1q[0:C, si : si + 1],
                    )
                    nc.scalar.activation(
                        sq[C:128], ps[C:128], AF.Square,
                        accum_out=st1q[C:128, si : si + 1],
                    )
                # shift copies for this chunk's rows
                rr = CHT * RT * ch
                nr = CHT * RT if ch < NCH - 1 else CHT * RT + 1
                nc.gpsimd.dma_start(
                    out=pA[C:128, rr : rr + nr, :],
                    in_=pA[0:C, rr + 1 : rr + 1 + nr, :],
                )
                nc.gpsimd.dma_start(
                    out=pB[0:C, rr : rr + nr, :],
                    in_=pB[C:128, rr + 1 : rr + 1 + nr, :],
                )

        def emit_raw_out():
            SCC = 8
            for pair in (0, 1):
                pA_, pB_ = pan[2 * pair], pan[2 * pair + 1]
                for ch in range(H // SCC):
                    pr0 = 1 + SCC * ch
                    so = stg.tile([128, SCC, W], F32, tag="so", name="so")
                    nc.scalar.activation(
                        so[0:C, :, :],
                        pA_[0:C, pr0 : pr0 + SCC, 1 : 1 + W], AF.Copy,
                    )
                    nc.vector.tensor_scalar(
                        so[C:128, :, :],
                        pB_[C:128, pr0 : pr0 + SCC, 1 : 1 + W],
                        1.0, None, MULT,
                    )
                    nc.sync.dma_start(
                        out=out[2 * pair, :, SCC * ch : SCC * ch + SCC, :],
                        in_=so[0:C, :, :],
                    )
                    nc.gpsimd.dma_start(
                        out=out[2 * pair + 1, :, SCC * ch : SCC * ch + SCC, :],
                        in_=so[C:128, :, :],
                    )

        if phase == "A":
            emit_raw_out()

        # ================= BN1 stats + fold into w2 =================
        if phase == "A":
            return
        nc.vector.tensor_reduce(red1[:, 0:1], st1[:], AX, ADD)
        nc.vector.tensor_reduce(red1[:, 1:2], st1q[:], AX, ADD)
        tot1 = do_collective(red1, "cc1")
        bn_coeffs(tot1, 0, 1, m1, s1f[:, 0:1], m1[:, 6:7])  # s1, b1
        nc.vector.tensor_scalar(b1c[:], m1[0:C, 6:7], 1.0, None, MULT)
        nc.vector.tensor_scalar(w2sA[:], w2A[:], s1f[:, 0:1], None, MULT)
        nc.vector.tensor_scalar(w2sB[:], w2B[:], s1f[:, 0:1], None, MULT)
        nc.vector.tensor_scalar(w2sS[:], w2S[:], s1f[:, 0:1], None, MULT)
        nc.vector.tensor_scalar(w2sQ[:], w2Q[:], s1f[:, 0:1], None, MULT)

        # bias field: tv[o, ky, kx] = sum_i w2[o,i,ky,kx] * b1[i]
        pstv = psb.tile([C, 16], F32, tag="pstv")
        for tap in range(9):
            nc.tensor.matmul(
                pstv[:, tap : tap + 1], w2T[:, tap, :], b1c[:, 0:1],
                start=True, stop=True, tile_position=(0, 0),
            )
        nc.scalar.activation(tv[:, :, :], pstv[:, 0:9], AF.Copy)
        # class sums (all [C,1]):
        nc.vector.tensor_reduce(m2[0:C, 0:1], tv[:, :, :], AXY, ADD)  # Csum
        nc.vector.tensor_reduce(m2[0:C, 1:2], tv[:, 0:1, :], AXY, ADD)  # dT
        nc.vector.tensor_reduce(m2[0:C, 2:3], tv[:, 2:3, :], AXY, ADD)  # dB
        nc.vector.tensor_reduce(m2[0:C, 3:4], tv[:, :, 0:1], AXY, ADD)  # dL
        nc.vector.tensor_reduce(m2[0:C, 4:5], tv[:, :, 2:3], AXY, ADD)  # dR
        # fix rows: fixT = -dT everywhere; corners -dT-dL+T00 / -dT-dR+T02
        nc.vector.tensor_scalar(
            fixT[0:C, :, :], ones[:], m2[0:C, 1:2], -1.0, MULT, op1=MULT
        )
        nc.vector.tensor_scalar(
            fixB[0:C, :, :], ones[:], m2[0:C, 2:3], -1.0, MULT, op1=MULT
        )
        # corner deltas: m2[0:C,5] = T00 - dL etc; then add -dT
        nc.vector.tensor_tensor(m2[0:C, 5:6], tv[:, 0, 0:1], m2[0:C, 3:4], SUB)
        nc.vector.tensor_tensor(
            fixT[0:C, 0, 0:1], m2[0:C, 5:6], m2[0:C, 1:2], SUB
        )
        nc.vector.tensor_tensor(m2[0:C, 5:6], tv[:, 0, 2:3], m2[0:C, 4:5], SUB)
        nc.vector.tensor_tensor(
            fixT[0:C, 0, W - 1 : W], m2[0:C, 5:6], m2[0:C, 1:2], SUB
        )
        nc.vector.tensor_tensor(m2[0:C, 5:6], tv[:, 2, 0:1], m2[0:C, 3:4], SUB)
        nc.vector.tensor_tensor(
            fixB[0:C, 0, 0:1], m2[0:C, 5:6], m2[0:C, 2:3], SUB
        )
        nc.vector.tensor_tensor(m2[0:C, 5:6], tv[:, 2, 2:3], m2[0:C, 4:5], SUB)
        nc.vector.tensor_tensor(
            fixB[0:C, 0, W - 1 : W], m2[0:C, 5:6], m2[0:C, 2:3], SUB
        )
        # sc: C, colL=-dL, colR=-dR, corr2
        nc.vector.tensor_scalar(sc[0:C, 0:1], m2[0:C, 0:1], 1.0, None, MULT)
        nc.vector.tensor_scalar(sc[0:C, 1:2], m2[0:C, 3:4], -1.0, None, MULT)
        nc.vector.tensor_scalar(sc[0:C, 2:3], m2[0:C, 4:5], -1.0, None, MULT)
        # corr2 = 2*( sum(fixT) + sum(fixB) + (H-2)*(colL+colR) )
        nc.vector.tensor_reduce(m2[0:C, 5:6], fixT[0:C, :, :], AXY, ADD)
        nc.vector.tensor_reduce(m2[0:C, 6:7], fixB[0:C, :, :], AXY, ADD)
        nc.vector.tensor_tensor(m2[0:C, 7:8], sc[0:C, 1:2], sc[0:C, 2:3], ADD)
        nc.vector.tensor_scalar(m2[0:C, 7:8], m2[0:C, 7:8], float(H - 2), None, MULT)
        nc.vector.tensor_tensor(m2[0:C, 5:6], m2[0:C, 5:6], m2[0:C, 6:7], ADD)
        nc.vector.tensor_tensor(m2[0:C, 5:6], m2[0:C, 5:6], m2[0:C, 7:8], ADD)
        nc.vector.tensor_scalar(sc[0:C, 3:4], m2[0:C, 5:6], 2.0, None, MULT)
        # duplicate to upper half
        nc.sync.dma_start(out=fixT[C:128, :, :], in_=fixT[0:C, :, :])
        nc.sync.dma_start(out=fixB[C:128, :, :], in_=fixB[0:C, :, :])
        nc.sync.dma_start(out=sc[C:128, 0:4], in_=sc[0:C, 0:4])

        # ================= Phase B: conv2 + residual =================
        # Evictions lag the matmuls by one tile: evict(t-1) overwrites padded
        # row 4t (o1 -> o3 in place), which MM(t) still reads as o1. Emitting
        # MM(t) first makes the framework order the overwrite after the read.
        for pair in (0, 1):
            pA, pB = pan[2 * pair], pan[2 * pair + 1]

            def evict2(t, ps, xr, lr, pA=pA, pB=pB, pair=pair):
                si = NT * pair + t
                tmp = sqp.tile([128, RT, W], BF16, tag="sq", name="tmp")
                nc.vector.tensor_tensor(
                    tmp[:], ps[:], xr[:, lr : lr + RT, :], ADD
                )
                nc.vector.tensor_scalar(
                    pA[0:C, 1 + RT * t : 1 + RT * t + RT, 1 : 1 + W],
                    tmp[0:C], sc[0:C, 0:1], 0.0, SUB, op1=ADD,
                    accum_out=st2[0:C, si : si + 1],
                )
                nc.vector.tensor_scalar(
                    pB[C:128, 1 + RT * t : 1 + RT * t + RT, 1 : 1 + W],
                    tmp[C:128], sc[C:128, 0:1], 0.0, SUB, op1=ADD,
                    accum_out=st2[C:128, si : si + 1],
                )

            def fix2(ch, pA=pA, pB=pB, pair=pair):
                pr0 = 1 + SCH * ch
                for pp, lo, hi in ((pA, 0, C), (pB, C, 128)):
                    if ch == 0:
                        nc.vector.tensor_tensor(
                            pp[lo:hi, 1:2, 1 : 1 + W],
                            pp[lo:hi, 1:2, 1 : 1 + W],
                            fixT[lo:hi, :, :], ADD,
                        )
                    if ch == H // SCH - 1:
                        nc.vector.tensor_tensor(
                            pp[lo:hi, HP - 2 : HP - 1, 1 : 1 + W],
                            pp[lo:hi, HP - 2 : HP - 1, 1 : 1 + W],
                            fixB[lo:hi, :, :], ADD,
                        )
                    ra = pr0 + 1 if ch == 0 else pr0
                    rb = pr0 + SCH - 1 if ch == H // SCH - 1 else pr0 + SCH
                    nc.vector.tensor_scalar(
                        pp[lo:hi, ra:rb, 1:2],
                        pp[lo:hi, ra:rb, 1:2],
                        sc[lo:hi, 1:2], None, ADD,
                    )
                    nc.vector.tensor_scalar(
                        pp[lo:hi, ra:rb, W : W + 1],
                        pp[lo:hi, ra:rb, W : W + 1],
                        sc[lo:hi, 2:3], None, ADD,
                    )
                qi = (H // SCH) * pair + ch
                sq2 = stg.tile([128, SCH, W], BF16, tag="so", name="sq2")
                nc.scalar.activation(
                    sq2[0:C, :, :],
                    pA[0:C, pr0 : pr0 + SCH, 1 : 1 + W],
                    AF.Square,
                    accum_out=st2q[0:C, qi : qi + 1],
                )
                nc.scalar.activation(
                    sq2[C:128, :, :],
                    pB[C:128, pr0 : pr0 + SCH, 1 : 1 + W],
                    AF.Square,
                    accum_out=st2q[C:128, qi : qi + 1],
                )

            pend = None
            for ch in range(H // SCH):
                xrs = []
                for hh in (0, 1):
                    xr_ = xrp.tile([128, SCH // 2, W], BF16, tag="xr", name="xr_")
                    r0x = SCH * ch + (SCH // 2) * hh
                    nc.sync.dma_start(
                        out=xr_[:], in_=xR[pair][:, r0x : r0x + SCH // 2, :]
                    )
                    xrs.append(xr_)
                # Q chunks: [o1 col-shifted | o1 plain], copied from the
                # panels' plain halves before the in-place o3 overwrite
                qr0 = SCH * ch + 2
                qts = []
                for i01 in (0, 1):
                    qt = qp.tile([128, SCH, WP], BF16, tag="q", name="qt")
                    pp = pA if i01 == 0 else pB
                    src_half = pp[0:C] if i01 == 0 else pp[C:128]
                    nc.sync.dma_start(
                        out=qt[0:C, :, 0 : WP - 1],
                        in_=src_half[:, qr0 : qr0 + SCH, 1:WP],
                    )
                    nc.gpsimd.dma_start(
                        out=qt[C:128],
                        in_=src_half[:, qr0 : qr0 + SCH, :],
                    )
                    qts.append(qt)
                for tl in range(SCH // RT):
                    t = (SCH // RT) * ch + tl
                    lr = RT * tl
                    ps = psum.tile([128, RT, W], F32, tag="ps", name="ps")
                    for i01 in (0, 1):
                        tp = (0, 64 * i01)
                        po = ps[64 * i01 : 64 * i01 + 64, :, :]
                        pp = pA if i01 == 0 else pB
                        wP = w2sA if i01 == 0 else w2sB
                        for kx in range(3):
                            nc.tensor.matmul(
                                po, wP[:, kx, :],
                                pp[:, RT * t : RT * t + RT, kx : kx + W],
                                start=(kx == 0), stop=False, tile_position=tp,
                            )
                        nc.tensor.matmul(
                            po, w2sQ[:, :],
                            qts[i01][:, lr : lr + RT, 0:W],
                            start=False, stop=False, tile_position=tp,
                        )
                        nc.tensor.matmul(
                            po,
                            w2sS[64 * i01 : 64 * i01 + 64, 2, :],
                            pp[
                                64 * i01 : 64 * i01 + 64,
                                RT * t + 2 : RT * t + 2 + RT,
                                2 : 2 + W,
                            ],
                            start=False, stop=True,
                            tile_position=(64 * i01, 64 * i01),
                        )
                    if pend is not None:
                        evict2(*pend)
                    pend = (t, ps, xrs[tl // 2], RT * (tl % 2))
                if ch > 0:
                    fix2(ch - 1)
            evict2(*pend)
            fix2(H // SCH - 1)

        if phase == "B":
            emit_raw_out()

        # ================= BN2 stats =================
        if phase == "B":
            return
        nc.vector.tensor_reduce(red2[:, 0:1], st2[:], AX, ADD)
        nc.vector.tensor_scalar(
            red2[:, 0:1], red2[:, 0:1], sc[:, 3:4], None, ADD
        )
        nc.vector.tensor_reduce(red2[:, 1:2], st2q[:], AX, ADD)
        tot2 = do_collective(red2, "cc2")
        bn_coeffs(tot2, 2, 3, m2, sc[:, 4:5], sc[:, 5:6])  # s2, b2

        # ================= Phase C: BN2 apply + store =================
        SCC = 8
        for pair in (0, 1):
            pA, pB = pan[2 * pair], pan[2 * pair + 1]
            for ch in range(H // SCC):
                pr0 = 1 + SCC * ch
                so = stg.tile([128, SCC, W], F32, tag="so")
                nc.scalar.activation(
                    so[0:C, :, :],
                    pA[0:C, pr0 : pr0 + SCC, 1 : 1 + W],
                    AF.Identity,
                    bias=sc[0:C, 5:6], scale=sc[0:C, 4:5],
                )
                nc.vector.tensor_scalar(
                    so[C:128, :, :],
                    pB[C:128, pr0 : pr0 + SCC, 1 : 1 + W],
                    sc[C:128, 4:5], sc[C:128, 5:6], MULT, op1=ADD,
                )
                nc.sync.dma_start(
                    out=out[2 * pair, :, SCC * ch : SCC * ch + SCC, :],
                    in_=so[0:C, :, :],
                )
                nc.gpsimd.dma_start(
                    out=out[2 * pair + 1, :, SCC * ch : SCC * ch + SCC, :],
                    in_=so[C:128, :, :],
                )

    with tile.TileContext(nc) as tc, ExitStack() as ctx:
        _emit(tc, ctx)
    nc.finalize()
    return nc


_NC_CACHE = {}


def _prep_inputs(inputs):
    x = np.asarray(inputs["x"], dtype=np.float32)
    w1 = np.asarray(inputs["w1"], dtype=np.float32)
    w2 = np.asarray(inputs["w2"], dtype=np.float32)
    g1 = np.asarray(inputs["bn1_gamma"], dtype=np.float32)
    b1 = np.asarray(inputs["bn1_beta"], dtype=np.float32)
    g2 = np.asarray(inputs["bn2_gamma"], dtype=np.float32)
    b2 = np.asarray(inputs["bn2_beta"], dtype=np.float32)
    bf = ml_dtypes.bfloat16

    xpad = np.zeros((N_IMG, C, HP, WP), np.float32)
    xpad[:, :, 1 : 1 + H, 1 : 1 + W] = x
    sh10 = np.zeros_like(xpad)
    sh10[:, :, 0 : HP - 1, :] = xpad[:, :, 1:HP, :]
    sh01 = np.zeros_like(xpad)
    sh01[:, :, :, 0 : WP - 1] = xpad[:, :, :, 1:WP]

    xP = np.concatenate([xpad, sh10], axis=1).astype(bf)   # [32, 128, HP, WP]
    xQ = np.concatenate([sh01, xpad], axis=1).astype(bf)
    xRf = x.reshape(N_IMG // 2, 2 * C, H, W).astype(bf)     # [16, 128, H, W]

    w1t = np.ascontiguousarray(w1.transpose(1, 2, 3, 0))   # [i, ky, kx, o]
    w2t = np.ascontiguousarray(w2.transpose(1, 2, 3, 0))
    w1P = np.concatenate([w1t[:, 0], w1t[:, 1]], axis=0).astype(bf)
    w1Q = np.concatenate([w1t[:, 2, 1], w1t[:, 2, 0]], axis=0).astype(bf)
    w1S = np.ascontiguousarray(w1t[:, 2, 2]).astype(bf)
    w2A = np.concatenate([w2t[:, 0], w2t[:, 1]], axis=0).astype(bf)
    w2B = np.concatenate([w2t[:, 1], w2t[:, 0]], axis=0).astype(bf)
    w2Sv = np.concatenate([w2t[:, 2], w2t[:, 2]], axis=0).astype(bf)
    w2T = np.ascontiguousarray(w2t.reshape(C, 9, C)).astype(bf)
    w2Qv = np.concatenate([w2t[:, 2, 1], w2t[:, 2, 0]], axis=0).astype(bf)
    prmv = np.tile(np.stack([g1, b1, g2, b2], axis=1), (2, 1)).astype(np.float32)

    in_maps = []
    for k in range(N_CORES):
        in_maps.append({
            "xP": np.ascontiguousarray(xP[4 * k : 4 * k + 4]),
            "xQ": np.ascontiguousarray(xQ[4 * k : 4 * k + 4]),
            "xR": np.ascontiguousarray(xRf[2 * k : 2 * k + 2]),
            "w1Pd": w1P, "w1Qd": w1Q, "w1Sd": w1S,
            "w2Ad": w2A, "w2Bd": w2B, "w2Sd": w2Sv, "w2Td": w2T,
            "w2Qd": w2Qv,
            "prmd": prmv,
        })
    return in_maps


def kernel(**inputs):
    ph = os.environ.get("KERNEL_PH", "")
    if ph not in _NC_CACHE:
        _NC_CACHE[ph] = _build_bass(phase=ph)
    nc = _NC_CACHE[ph]
    in_maps = _prep_inputs(inputs)
    trace = bool(int(os.environ.get("KERNEL_TRACE", "0")))
    res = run_bass_kernel_spmd(
        nc, in_maps, core_ids=list(range(N_CORES)), trace=trace
    )
    if trace:
        kernel.last_exec_time_ns = res.exec_time_ns
        kernel.last_results = res
    out = np.concatenate([r["out"] for r in res.results], axis=0)
    return out.astype(np.float32)


if __name__ == "__main__":
    nc = _build_bass()
    print("build ok")


# revision 5
# speedup vs baseline: 1.0059x; 1.0059x over previous
"""Trainium2 Bass kernel v2 for ResNet BasicBlock (conv3x3-BN-conv3x3-+x-BN).

Data-parallel over 8 cores (4 images each). Per core:

- Conv as tap-packed matmuls: each image is stored as 128-partition "panels"
  [plain | shifted] so a single K=128 matmul covers TWO conv taps at once.
  conv1: 5 matmuls / image / spatial tile (3 row-pairs + 1 col-pair + 1
  single), conv2: 6 (3 row-pairs + 3 singles). Two images run per tile-step
  on the two PE column halves (tile_position).
- BN1 is folded into conv2: scale into w2 (w2s = w2 * s1 per input channel),
  bias via a constant C[o] = sum_i,tap w2[o,i,tap]*b1[i] subtracted at
  eviction plus small border fixups (conv padding makes the bias field
  non-constant only at the image border).
- conv2 writes (psum - C + x_residual) in place into the panels' plain
  halves; BN2 stats accumulate via fused accum_out on the eviction ops;
  batch stats all-reduced across cores ([64,2] f32 AllReduce per BN).
- Phase C applies BN2 affine (ACT half / DVE half) and streams out f32.
"""

import os
import sys

sys.path.insert(0, "/opt/trn_rl_repo")

import numpy as np
import ml_dtypes

from contextlib import ExitStack

from concourse import bacc, bass, mybir, tile
from concourse.bass_utils import run_bass_kernel_spmd

F32 = mybir.dt.float32
BF16 = mybir.dt.bfloat16
ADD = mybir.AluOpType.add
MULT = mybir.AluOpType.mult
SUB = mybir.AluOpType.subtract
AF = mybir.ActivationFunctionType
AX = mybir.AxisListType.X
AXY = mybir.AxisListType.XY

N_CORES = 8
N_IMG = 32
C = 64
H = W = 128
HP = WP = 130
RT = 4            # rows per spatial tile
NT = H // RT      # 32 tiles per image pair
CHT = 4           # tiles per x chunk
NCH = NT // CHT   # 8 chunks
CHR = CHT * RT + 2  # 18 padded rows per P chunk window
SCH = 16          # rows per xres / phase-C / sumsq2 chunk
NHW = N_IMG * H * W
EPS = 1e-5


def _build_bass(n_cores=N_CORES, nhw=NHW, phase=""):
    nc = bacc.Bacc(
        "TRN2", target_bir_lowering=False, debug=False, num_devices=n_cores
    )

    xP = nc.dram_tensor("xP", [4, 128, HP, WP], BF16, kind="ExternalInput")
    xQ = nc.dram_tensor("xQ", [4, 128, HP, WP], BF16, kind="ExternalInput")
    xR = nc.dram_tensor("xR", [2, 128, H, W], BF16, kind="ExternalInput")
    w1Pd = nc.dram_tensor("w1Pd", [128, 3, C], BF16, kind="ExternalInput")
    w1Qd = nc.dram_tensor("w1Qd", [128, C], BF16, kind="ExternalInput")
    w1Sd = nc.dram_tensor("w1Sd", [C, C], BF16, kind="ExternalInput")
    w2Ad = nc.dram_tensor("w2Ad", [128, 3, C], BF16, kind="ExternalInput")
    w2Bd = nc.dram_tensor("w2Bd", [128, 3, C], BF16, kind="ExternalInput")
    w2Sd = nc.dram_tensor("w2Sd", [128, 3, C], BF16, kind="ExternalInput")
    w2Td = nc.dram_tensor("w2Td", [C, 9, C], BF16, kind="ExternalInput")
    w2Qd = nc.dram_tensor("w2Qd", [128, C], BF16, kind="ExternalInput")
    prmd = nc.dram_tensor("prmd", [128, 4], F32, kind="ExternalInput")
    out = nc.dram_tensor("out", [4, C, H, W], F32, kind="ExternalOutput")

    rg8 = [list(range(n_cores))]

    def _emit(tc, ctx):
        const = ctx.enter_context(tc.tile_pool(name="const", bufs=1))
        panels = ctx.enter_context(tc.tile_pool(name="panels", bufs=1))
        xpp = ctx.enter_context(tc.tile_pool(name="xpp", bufs=4))
        xqp = ctx.enter_context(tc.tile_pool(name="xqp", bufs=3))
        xrp = ctx.enter_context(tc.tile_pool(name="xrp", bufs=4))
        sqp = ctx.enter_context(tc.tile_pool(name="sqp", bufs=2))
        stat = ctx.enter_context(tc.tile_pool(name="stat", bufs=1))
        stg = ctx.enter_context(tc.tile_pool(name="stg", bufs=4))
        qp = ctx.enter_context(tc.tile_pool(name="qp", bufs=3))
        psum = ctx.enter_context(tc.tile_pool(name="psum", bufs=6, space="PSUM"))
        psb = ctx.enter_context(tc.tile_pool(name="psb", bufs=1, space="PSUM"))
        dram = ctx.enter_context(tc.tile_pool(name="dram", bufs=4, space="DRAM"))

        # ---- weights / params ----
        w1P = const.tile([128, 3, C], BF16, tag="w1P")
        w1Q = const.tile([128, C], BF16, tag="w1Q")
        w1S = const.tile([C, C], BF16, tag="w1S")
        w2A = const.tile([128, 3, C], BF16, tag="w2A")
        w2B = const.tile([128, 3, C], BF16, tag="w2B")
        w2S = const.tile([128, 3, C], BF16, tag="w2S")
        w2T = const.tile([C, 9, C], BF16, tag="w2T")
        w2Q = const.tile([128, C], BF16, tag="w2Q")
        prm = const.tile([128, 4], F32, tag="prm")
        for sb, d in ((w1P, w1Pd), (w1Q, w1Qd), (w1S, w1Sd), (w2A, w2Ad),
                      (w2B, w2Bd), (w2S, w2Sd), (w2T, w2Td), (w2Q, w2Qd),
                      (prm, prmd)):
            nc.scalar.dma_start(out=sb[:], in_=d[:])
        ones = const.tile([C, 1, W], BF16, tag="ones")
        nc.gpsimd.memset(ones[:], 1.0)

        # scaled conv2 weights (built after cc1)
        w2sA = const.tile([128, 3, C], BF16, tag="w2sA")
        w2sB = const.tile([128, 3, C], BF16, tag="w2sB")
        w2sS = const.tile([128, 3, C], BF16, tag="w2sS")
        w2sQ = const.tile([128, C], BF16, tag="w2sQ")

        # ---- persistent panels (conv1 out -> conv2 in -> o3) ----
        # pan[img]: [plain-img | shift(1,0)-img] for even img (A-layout),
        #           [shift(1,0)-img | plain-img] for odd img (B-layout).
        pan = [
            panels.tile([128, HP, WP], BF16, tag=f"pan{i}", name=f"pan{i}")
            for i in range(4)
        ]
        for p in pan:
            nc.gpsimd.memset(p[:, 0:1, :], 0.0)
            nc.gpsimd.memset(p[:, HP - 1 : HP, :], 0.0)
            nc.gpsimd.memset(p[:, :, 0:1], 0.0)
            nc.gpsimd.memset(p[:, :, WP - 1 : WP], 0.0)

        # ---- stats ----
        st1 = stat.tile([128, 2 * NT], F32, tag="st1")
        st1q = stat.tile([128, 2 * NT], F32, tag="st1q")
        st2 = stat.tile([128, 2 * NT], F32, tag="st2")
        st2q = stat.tile([128, 2 * H // SCH], F32, tag="st2q")
        red1 = stat.tile([128, 2], F32, tag="red1")
        red2 = stat.tile([128, 2], F32, tag="red2")

        # bias-field / coeff tiles
        fixT = stat.tile([128, 1, W], F32, tag="fixT")
        fixB = stat.tile([128, 1, W], F32, tag="fixB")
        sc = stat.tile([128, 8], F32, tag="sc")
        # sc cols: 0=C, 1=colL, 2=colR, 3=corr2, 4=s2, 5=b2
        m1 = stat.tile([128, 8], F32, tag="m1")
        m2 = stat.tile([128, 8], F32, tag="m2")
        tv = stat.tile([C, 3, 3], F32, tag="tv")
        s1f = stat.tile([128, 1], F32, tag="s1f")
        b1c = stat.tile([C, 1], BF16, tag="b1c")

        def bn_coeffs(tot, gcol, bcol, m, s_out, b_out):
            """tot [128,2] (sum, sumsq) -> scale/bias [128,1] f32 (dual-half)."""
            nc.vector.tensor_scalar(m[:, 0:2], tot[:, 0:2], 1.0 / nhw, None, MULT)
            nc.vector.tensor_tensor(m[:, 2:3], m[:, 0:1], m[:, 0:1], MULT)
            nc.vector.tensor_scalar(
                m[:, 3:4], m[:, 1:2], m[:, 2:3], EPS, SUB, op1=ADD
            )  # var + eps
            nc.vector.reciprocal(m[:, 6:7], m[:, 3:4])
            nc.scalar.activation(m[:, 4:5], m[:, 6:7], AF.Sqrt)
            nc.vector.tensor_tensor(
                s_out, prm[:, gcol : gcol + 1], m[:, 4:5], MULT
            )
            nc.vector.tensor_tensor(m[:, 5:6], m[:, 0:1], s_out, MULT)
            nc.vector.tensor_tensor(
                b_out, prm[:, bcol : bcol + 1], m[:, 5:6], SUB
            )

        def do_collective(src, cc_name):
            # AllGather the raw per-core [128,2] (sum, sumsq) stats, then
            # reduce over (core, partition-half) on-chip. An AllGather of
            # this size is ~2x cheaper than an AllReduce and needs no
            # pre-fold DMAs.
            cc_in = dram.tile([128, 2], F32, tag=cc_name + "i")
            cc_out = dram.tile([n_cores, 2, C, 2], F32, tag=cc_name + "o")
            nc.sync.dma_start(out=cc_in[:], in_=src[:])
            if os.environ.get("KERNEL_NOCC"):
                nc.sync.dma_start(out=cc_out[0, 0], in_=cc_in[0:C, :])
                nc.sync.dma_start(out=cc_out[0, 1], in_=cc_in[C:128, :])
                for k in range(1, n_cores):
                    nc.gpsimd.memset(cc_out[k], 0.0)
            else:
                nc.gpsimd.collective_compute(
                    "AllGather", mybir.AluOpType.bypass, replica_groups=rg8,
                    ins=[cc_in[:].opt()], outs=[cc_out[:].opt()],
                )
            g = stat.tile([128, 2, 2 * n_cores], F32, tag=cc_name + "g")
            gv = cc_out[:].rearrange("k h c s -> c s (k h)")
            nc.sync.dma_start(out=g[0:C], in_=gv)
            nc.scalar.dma_start(out=g[C:128], in_=gv)
            tot = stat.tile([128, 2], F32, tag=cc_name + "t")
            nc.vector.tensor_reduce(tot[:], g[:], AX, ADD)
            return tot

        # ================= Phase A: conv1 =================
        for pair in (0, 1):
            pA, pB = pan[2 * pair], pan[2 * pair + 1]
            for ch in range(NCH):
                r0 = CHT * RT * ch
                cps, cqs = [], []
                for i01 in (0, 1):
                    img = 2 * pair + i01
                    cp = xpp.tile([128, CHR, WP], BF16, tag="xp")
                    nc.sync.dma_start(out=cp[:], in_=xP[img][:, r0 : r0 + CHR, :])
                    cq = xqp.tile([128, CHR - 2, WP], BF16, tag="xq")
                    nc.sync.dma_start(
                        out=cq[:], in_=xQ[img][:, r0 + 2 : r0 + CHR, :]
                    )
                    cps.append(cp)
                    cqs.append(cq)
                for tl in range(CHT):
                    t = CHT * ch + tl
                    si = NT * pair + t
                    l0 = RT * tl
                    ps = psum.tile([128, RT, W], F32, tag="ps")
                    for i01 in (0, 1):
                        tp = (0, 64 * i01)
                        po = ps[64 * i01 : 64 * i01 + 64, :, :]
                        cp, cq = cps[i01], cqs[i01]
                        for kx in range(3):
                            nc.tensor.matmul(
                                po, w1P[:, kx, :],
                                cp[:, l0 : l0 + RT, kx : kx + W],
                                start=(kx == 0), stop=False, tile_position=tp,
                            )
                        nc.tensor.matmul(
                            po, w1Q[:, :], cq[:, l0 : l0 + RT, 0:W],
                            start=False, stop=False, tile_position=tp,
                        )
                        nc.tensor.matmul(
                            po, w1S[:, :],
                            cp[0:C, l0 + 2 : l0 + 2 + RT, 2 : 2 + W],
                            start=False, stop=True, tile_position=tp,
                        )
                    # evictions + stats (DVE copies, ACT squares from psum)
                    nc.vector.tensor_scalar(
                        pA[0:C, 1 + RT * t : 1 + RT * t + RT, 1 : 1 + W],
                        ps[0:C], 1.0, 0.0, MULT, op1=ADD,
                        accum_out=st1[0:C, si : si + 1],
                    )
                    nc.vector.tensor_scalar(
                        pB[C:128, 1 + RT * t : 1 + RT * t + RT, 1 : 1 + W],
                        ps[C:128], 1.0, 0.0, MULT, op1=ADD,
                        accum_out=st1[C:128, si : si + 1],
                    )
                    sq = sqp.tile([128, RT, W], BF16, tag="sq")
                    nc.scalar.activation(
                        sq[0:C], ps[0:C], AF.Square,
                        accum_out=st1q[0:C, si : si + 1],
                    )
                    nc.scalar.activation(
                        sq[C:128], ps[C:128], AF.Square,
                        accum_out=st1q[C:128, si : si + 1],
                    )
                # shift copies for this chunk's rows
                rr = CHT * RT * ch
                nr = CHT * RT if ch < NCH - 1 else CHT * RT + 1
                nc.gpsimd.dma_start(
                    out=pA[C:128, rr : rr + nr, :],
                    in_=pA[0:C, rr + 1 : rr + 1 + nr, :],
                )
                nc.gpsimd.dma_start(
                    out=pB[0:C, rr : rr + nr, :],
                    in_=pB[C:128, rr + 1 : rr + 1 + nr, :],
                )

        def emit_raw_out():
            SCC = 8
            for pair in (0, 1):
                pA_, pB_ = pan[2 * pair], pan[2 * pair + 1]
                for ch in range(H // SCC):
                    pr0 = 1 + SCC * ch
                    so = stg.tile([128, SCC, W], F32, tag="so", name="so")
                    nc.scalar.activation(
                        so[0:C, :, :],
                        pA_[0:C, pr0 : pr0 + SCC, 1 : 1 + W], AF.Copy,
                    )
                    nc.vector.tensor_scalar(
                        so[C:128, :, :],
                        pB_[C:128, pr0 : pr0 + SCC, 1 : 1 + W],
                        1.0, None, MULT,
                    )
                    nc.sync.dma_start(
                        out=out[2 * pair, :, SCC * ch : SCC * ch + SCC, :],
                        in_=so[0:C, :, :],
                    )
                    nc.gpsimd.dma_start(
                        out=out[2 * pair + 1, :, SCC * ch : SCC * ch + SCC, :],
                        in_=so[C:128, :, :],
                    )

        if phase == "A":
            emit_raw_out()

        # ================= BN1 stats + fold into w2 =================
        if phase == "A":
            return
        nc.vector.tensor_reduce(red1[:, 0:1], st1[:], AX, ADD)
        nc.vector.tensor_reduce(red1[:, 1:2], st1q[:], AX, ADD)
        tot1 = do_collective(red1, "cc1")
        bn_coeffs(tot1, 0, 1, m1, s1f[:, 0:1], m1[:, 6:7])  # s1, b1
        nc.vector.tensor_scalar(b1c[:], m1[0:C, 6:7], 1.0, None, MULT)
        nc.vector.tensor_scalar(w2sA[:], w2A[:], s1f[:, 0:1], None, MULT)
        nc.vector.tensor_scalar(w2sB[:], w2B[:], s1f[:, 0:1], None, MULT)
        nc.vector.tensor_scalar(w2sS[:], w2S[:], s1f[:, 0:1], None, MULT)
        nc.vector.tensor_scalar(w2sQ[:], w2Q[:], s1f[:, 0:1], None, MULT)

        # bias field: tv[o, ky, kx] = sum_i w2[o,i,ky,kx] * b1[i]
        pstv = psb.tile([C, 16], F32, tag="pstv")
        for tap in range(9):
            nc.tensor.matmul(
                pstv[:, tap : tap + 1], w2T[:, tap, :], b1c[:, 0:1],
                start=True, stop=True, tile_position=(0, 0),
            )
        nc.scalar.activation(tv[:, :, :], pstv[:, 0:9], AF.Copy)
        # class sums (all [C,1]):
        nc.vector.tensor_reduce(m2[0:C, 0:1], tv[:, :, :], AXY, ADD)  # Csum
        nc.vector.tensor_reduce(m2[0:C, 1:2], tv[:, 0:1, :], AXY, ADD)  # dT
        nc.vector.tensor_reduce(m2[0:C, 2:3], tv[:, 2:3, :], AXY, ADD)  # dB
        nc.vector.tensor_reduce(m2[0:C, 3:4], tv[:, :, 0:1], AXY, ADD)  # dL
        nc.vector.tensor_reduce(m2[0:C, 4:5], tv[:, :, 2:3], AXY, ADD)  # dR
        # fix rows: fixT = -dT everywhere; corners -dT-dL+T00 / -dT-dR+T02
        nc.vector.tensor_scalar(
            fixT[0:C, :, :], ones[:], m2[0:C, 1:2], -1.0, MULT, op1=MULT
        )
        nc.vector.tensor_scalar(
            fixB[0:C, :, :], ones[:], m2[0:C, 2:3], -1.0, MULT, op1=MULT
        )
        # corner deltas: m2[0:C,5] = T00 - dL etc; then add -dT
        nc.vector.tensor_tensor(m2[0:C, 5:6], tv[:, 0, 0:1], m2[0:C, 3:4], SUB)
        nc.vector.tensor_tensor(
            fixT[0:C, 0, 0:1], m2[0:C, 5:6], m2[0:C, 1:2], SUB
        )
        nc.vector.tensor_tensor(m2[0:C, 5:6], tv[:, 0, 2:3], m2[0:C, 4:5], SUB)
        nc.vector.tensor_tensor(
            fixT[0:C, 0, W - 1 : W], m2[0:C, 5:6], m2[0:C, 1:2], SUB
        )
        nc.vector.tensor_tensor(m2[0:C, 5:6], tv[:, 2, 0:1], m2[0:C, 3:4], SUB)
        nc.vector.tensor_tensor(
            fixB[0:C, 0, 0:1], m2[0:C, 5:6], m2[0:C, 2:3], SUB
        )
        nc.vector.tensor_tensor(m2[0:C, 5:6], tv[:, 2, 2:3], m2[0:C, 4:5], SUB)
        nc.vector.tensor_tensor(
            fixB[0:C, 0, W - 1 : W], m2[0:C, 5:6], m2[0:C, 2:3], SUB
        )
        # sc: C, colL=-dL, colR=-dR, corr2
        nc.vector.tensor_scalar(sc[0:C, 0:1], m2[0:C, 0:1], 1.0, None, MULT)
        nc.vector.tensor_scalar(sc[0:C, 1:2], m2[0:C, 3:4], -1.0, None, MULT)
        nc.vector.tensor_scalar(sc[0:C, 2:3], m2[0:C, 4:5], -1.0, None, MULT)
        # corr2 = 2*( sum(fixT) + sum(fixB) + (H-2)*(colL+colR) )
        nc.vector.tensor_reduce(m2[0:C, 5:6], fixT[0:C, :, :], AXY, ADD)
        nc.vector.tensor_reduce(m2[0:C, 6:7], fixB[0:C, :, :], AXY, ADD)
        nc.vector.tensor_tensor(m2[0:C, 7:8], sc[0:C, 1:2], sc[0:C, 2:3], ADD)
        nc.vector.tensor_scalar(m2[0:C, 7:8], m2[0:C, 7:8], float(H - 2), None, MULT)
        nc.vector.tensor_tensor(m2[0:C, 5:6], m2[0:C, 5:6], m2[0:C, 6:7], ADD)
        nc.vector.tensor_tensor(m2[0:C, 5:6], m2[0:C, 5:6], m2[0:C, 7:8], ADD)
        nc.vector.tensor_scalar(sc[0:C, 3:4], m2[0:C, 5:6], 2.0, None, MULT)
        # duplicate to upper half
        nc.sync.dma_start(out=fixT[C:128, :, :], in_=fixT[0:C, :, :])
        nc.sync.dma_start(out=fixB[C:128, :, :], in_=fixB[0:C, :, :])
        nc.sync.dma_start(out=sc[C:128, 0:4], in_=sc[0:C, 0:4])

        # ================= Phase B: conv2 + residual =================
        # Evictions lag the matmuls by one tile: evict(t-1) overwrites padded
        # row 4t (o1 -> o3 in place), which MM(t) still reads as o1. Emitting
        # MM(t) first makes the framework order the overwrite after the read.
        for pair in (0, 1):
            pA, pB = pan[2 * pair], pan[2 * pair + 1]

            def evict2(t, ps, xr, lr, pA=pA, pB=pB, pair=pair):
                si = NT * pair + t
                tmp = sqp.tile([128, RT, W], BF16, tag="sq", name="tmp")
                nc.vector.tensor_tensor(
                    tmp[:], ps[:], xr[:, lr : lr + RT, :], ADD
                )
                nc.vector.tensor_scalar(
                    pA[0:C, 1 + RT * t : 1 + RT * t + RT, 1 : 1 + W],
                    tmp[0:C], sc[0:C, 0:1], 0.0, SUB, op1=ADD,
                    accum_out=st2[0:C, si : si + 1],
                )
                nc.vector.tensor_scalar(
                    pB[C:128, 1 + RT * t : 1 + RT * t + RT, 1 : 1 + W],
                    tmp[C:128], sc[C:128, 0:1], 0.0, SUB, op1=ADD,
                    accum_out=st2[C:128, si : si + 1],
                )

            def fix2(ch, pA=pA, pB=pB, pair=pair):
                pr0 = 1 + SCH * ch
                for pp, lo, hi in ((pA, 0, C), (pB, C, 128)):
                    if ch == 0:
                        nc.vector.tensor_tensor(
                            pp[lo:hi, 1:2, 1 : 1 + W],
                            pp[lo:hi, 1:2, 1 : 1 + W],
                            fixT[lo:hi, :, :], ADD,
                        )
                    if ch == H // SCH - 1:
                        nc.vector.tensor_tensor(
                            pp[lo:hi, HP - 2 : HP - 1, 1 : 1 + W],
                            pp[lo:hi, HP - 2 : HP - 1, 1 : 1 + W],
                            fixB[lo:hi, :, :], ADD,
                        )
                    ra = pr0 + 1 if ch == 0 else pr0
                    rb = pr0 + SCH - 1 if ch == H // SCH - 1 else pr0 + SCH
                    nc.vector.tensor_scalar(
                        pp[lo:hi, ra:rb, 1:2],
                        pp[lo:hi, ra:rb, 1:2],
                        sc[lo:hi, 1:2], None, ADD,
                    )
                    nc.vector.tensor_scalar(
                        pp[lo:hi, ra:rb, W : W + 1],
                        pp[lo:hi, ra:rb, W : W + 1],
                        sc[lo:hi, 2:3], None, ADD,
                    )
                qi = (H // SCH) * pair + ch
                sq2 = stg.tile([128, SCH, W], BF16, tag="so", name="sq2")
                nc.scalar.activation(
                    sq2[0:C, :, :],
                    pA[0:C, pr0 : pr0 + SCH, 1 : 1 + W],
                    AF.Square,
                    accum_out=st2q[0:C, qi : qi + 1],
                )
                nc.scalar.activation(
                    sq2[C:128, :, :],
                    pB[C:128, pr0 : pr0 + SCH, 1 : 1 + W],
                    AF.Square,
                    accum_out=st2q[C:128, qi : qi + 1],
                )

            pend = None
            for ch in range(H // SCH):
                xrs = []
                for hh in (0, 1):
                    xr_ = xrp.tile([128, SCH // 2, W], BF16, tag="xr", name="xr_")
                    r0x = SCH * ch + (SCH // 2) * hh
                    nc.sync.dma_start(
                        out=xr_[:], in_=xR[pair][:, r0x : r0x + SCH // 2, :]
                    )
                    xrs.append(xr_)
                # Q chunks: [o1 col-shifted | o1 plain], copied from the
                # panels' plain halves before the in-place o3 overwrite
                qr0 = SCH * ch + 2
                qts = []
                for i01 in (0, 1):
                    qt = qp.tile([128, SCH, WP], BF16, tag="q", name="qt")
                    pp = pA if i01 == 0 else pB
                    src_half = pp[0:C] if i01 == 0 else pp[C:128]
                    nc.sync.dma_start(
                        out=qt[0:C, :, 0 : WP - 1],
                        in_=src_half[:, qr0 : qr0 + SCH, 1:WP],
                    )
                    nc.gpsimd.dma_start(
                        out=qt[C:128],
                        in_=src_half[:, qr0 : qr0 + SCH, :],
                    )
                    qts.append(qt)
                for tl in range(SCH // RT):
                    t = (SCH // RT) * ch + tl
                    lr = RT * tl
                    ps = psum.tile([128, RT, W], F32, tag="ps", name="ps")
                    for i01 in (0, 1):
                        tp = (0, 64 * i01)
                        po = ps[64 * i01 : 64 * i01 + 64, :, :]
                        pp = pA if i01 == 0 else pB
                        wP = w2sA if i01 == 0 else w2sB
                        for kx in range(3):
                            nc.tensor.matmul(
                                po, wP[:, kx, :],
                                pp[:, RT * t : RT * t + RT, kx : kx + W],
                                start=(kx == 0), stop=False, tile_position=tp,
                            )
                        nc.tensor.matmul(
                            po, w2sQ[:, :],
                            qts[i01][:, lr : lr + RT, 0:W],
                            start=False, stop=False, tile_position=tp,
                        )
                        nc.tensor.matmul(
                            po,
                            w2sS[64 * i01 : 64 * i01 + 64, 2, :],
                            pp[
                                64 * i01 : 64 * i01 + 64,
                                RT * t + 2 : RT * t + 2 + RT,
                                2 : 2 + W,
                            ],
                            start=False, stop=True,
                            tile_position=(64 * i01, 64 * i01),
                        )
                    if pend is not None:
                        evict2(*pend)
                    pend = (t, ps, xrs[tl // 2], RT * (tl % 2))
                if ch > 0:
                    fix2(ch - 1)
            evict2(*pend)
            fix2(H // SCH - 1)

        if phase == "B":
            emit_raw_out()

        # ================= BN2 stats =================
        if phase == "B":
            return
        nc.vector.tensor_reduce(red2[:, 0:1], st2[:], AX, ADD)
        nc.vector.tensor_scalar(
            red2[:, 0:1], red2[:, 0:1], sc[:, 3:4], None, ADD
        )
        nc.vector.tensor_reduce(red2[:, 1:2], st2q[:], AX, ADD)
        tot2 = do_collective(red2, "cc2")
        bn_coeffs(tot2, 2, 3, m2, sc[:, 4:5], sc[:, 5:6])  # s2, b2

        # ================= Phase C: BN2 apply + store =================
        SCC = 8
        for ch in range(H // SCC):
            for pair in (0, 1):
                pA, pB = pan[2 * pair], pan[2 * pair + 1]
                pr0 = 1 + SCC * ch
                so = stg.tile([128, SCC, W], F32, tag="so")
                nc.scalar.activation(
                    so[0:C, :, :],
                    pA[0:C, pr0 : pr0 + SCC, 1 : 1 + W],
                    AF.Identity,
                    bias=sc[0:C, 5:6], scale=sc[0:C, 4:5],
                )
                nc.vector.tensor_scalar(
                    so[C:128, :, :],
                    pB[C:128, pr0 : pr0 + SCC, 1 : 1 + W],
                    sc[C:128, 4:5], sc[C:128, 5:6], MULT, op1=ADD,
                )
                nc.sync.dma_start(
                    out=out[2 * pair, :, SCC * ch : SCC * ch + SCC, :],
                    in_=so[0:C, :, :],
                )
                nc.gpsimd.dma_start(
                    out=out[2 * pair + 1, :, SCC * ch : SCC * ch + SCC, :],
                    in_=so[C:128, :, :],
                )

    with tile.TileContext(nc) as tc, ExitStack() as ctx:
        _emit(tc, ctx)
    nc.finalize()
    return nc


_NC_CACHE = {}


def _prep_inputs(inputs):
    x = np.asarray(inputs["x"], dtype=np.float32)
    w1 = np.asarray(inputs["w1"], dtype=np.float32)
    w2 = np.asarray(inputs["w2"], dtype=np.float32)
    g1 = np.asarray(inputs["bn1_gamma"], dtype=np.float32)
    b1 = np.asarray(inputs["bn1_beta"], dtype=np.float32)
    g2 = np.asarray(inputs["bn2_gamma"], dtype=np.float32)
    b2 = np.asarray(inputs["bn2_beta"], dtype=np.float32)
    bf = ml_dtypes.bfloat16

    xpad = np.zeros((N_IMG, C, HP, WP), np.float32)
    xpad[:, :, 1 : 1 + H, 1 : 1 + W] = x
    sh10 = np.zeros_like(xpad)
    sh10[:, :, 0 : HP - 1, :] = xpad[:, :, 1:HP, :]
    sh01 = np.zeros_like(xpad)
    sh01[:, :, :, 0 : WP - 1] = xpad[:, :, :, 1:WP]

    xP = np.concatenate([xpad, sh10], axis=1).astype(bf)   # [32, 128, HP, WP]
    xQ = np.concatenate([sh01, xpad], axis=1).astype(bf)
    xRf = x.reshape(N_IMG // 2, 2 * C, H, W).astype(bf)     # [16, 128, H, W]

    w1t = np.ascontiguousarray(w1.transpose(1, 2, 3, 0))   # [i, ky, kx, o]
    w2t = np.ascontiguousarray(w2.transpose(1, 2, 3, 0))
    w1P = np.concatenate([w1t[:, 0], w1t[:, 1]], axis=0).astype(bf)
    w1Q = np.concatenate([w1t[:, 2, 1], w1t[:, 2, 0]], axis=0).astype(bf)
    w1S = np.ascontiguousarray(w1t[:, 2, 2]).astype(bf)
    w2A = np.concatenate([w2t[:, 0], w2t[:, 1]], axis=0).astype(bf)
    w2B = np.concatenate([w2t[:, 1], w2t[:, 0]], axis=0).astype(bf)
    w2Sv = np.concatenate([w2t[:, 2], w2t[:, 2]], axis=0).astype(bf)
    w2T = np.ascontiguousarray(w2t.reshape(C, 9, C)).astype(bf)
    w2Qv = np.concatenate([w2t[:, 2, 1], w2t[:, 2, 0]], axis=0).astype(bf)
    prmv = np.tile(np.stack([g1, b1, g2, b2], axis=1), (2, 1)).astype(np.float32)

    in_maps = []
    for k in range(N_CORES):
        in_maps.append({
            "xP": np.ascontiguousarray(xP[4 * k : 4 * k + 4]),
            "xQ": np.ascontiguousarray(xQ[4 * k : 4 * k + 4]),
            "xR": np.ascontiguousarray(xRf[2 * k : 2 * k + 2]),
            "w1Pd": w1P, "w1Qd": w1Q, "w1Sd": w1S,
            "w2Ad": w2A, "w2Bd": w2B, "w2Sd": w2Sv, "w2Td": w2T,
            "w2Qd": w2Qv,
            "prmd": prmv,
        })
    return in_maps


def kernel(**inputs):
    ph = os.environ.get("KERNEL_PH", "")
    if ph not in _NC_CACHE:
        _NC_CACHE[ph] = _build_bass(phase=ph)
    nc = _NC_CACHE[ph]
    in_maps = _prep_inputs(inputs)
    trace = bool(int(os.environ.get("KERNEL_TRACE", "0")))
    res = run_bass_kernel_spmd(
        nc, in_maps, core_ids=list(range(N_CORES)), trace=trace
    )
    if trace:
        kernel.last_exec_time_ns = res.exec_time_ns
        kernel.last_results = res
    out = np.concatenate([r["out"] for r in res.results], axis=0)
    return out.astype(np.float32)


if __name__ == "__main__":
    nc = _build_bass()
    print("build ok")


# revision 6
# speedup vs baseline: 1.0064x; 1.0005x over previous
"""Trainium2 Bass kernel v2 for ResNet BasicBlock (conv3x3-BN-conv3x3-+x-BN).

Data-parallel over 8 cores (4 images each). Per core:

- Conv as tap-packed matmuls: each image is stored as 128-partition "panels"
  [plain | shifted] so a single K=128 matmul covers TWO conv taps at once.
  conv1: 5 matmuls / image / spatial tile (3 row-pairs + 1 col-pair + 1
  single), conv2: 6 (3 row-pairs + 3 singles). Two images run per tile-step
  on the two PE column halves (tile_position).
- BN1 is folded into conv2: scale into w2 (w2s = w2 * s1 per input channel),
  bias via a constant C[o] = sum_i,tap w2[o,i,tap]*b1[i] subtracted at
  eviction plus small border fixups (conv padding makes the bias field
  non-constant only at the image border).
- conv2 writes (psum - C + x_residual) in place into the panels' plain
  halves; BN2 stats accumulate via fused accum_out on the eviction ops;
  batch stats all-reduced across cores ([64,2] f32 AllReduce per BN).
- Phase C applies BN2 affine (ACT half / DVE half) and streams out f32.
"""

import os
import sys

sys.path.insert(0, "/opt/trn_rl_repo")

import numpy as np
import ml_dtypes

from contextlib import ExitStack

from concourse import bacc, bass, mybir, tile
from concourse.bass_utils import run_bass_kernel_spmd

F32 = mybir.dt.float32
BF16 = mybir.dt.bfloat16
ADD = mybir.AluOpType.add
MULT = mybir.AluOpType.mult
SUB = mybir.AluOpType.subtract
AF = mybir.ActivationFunctionType
AX = mybir.AxisListType.X
AXY = mybir.AxisListType.XY

N_CORES = 8
N_IMG = 32
C = 64
H = W = 128
HP = WP = 130
RT = 4            # rows per spatial tile
NT = H // RT      # 32 tiles per image pair
CHT = 4           # tiles per x chunk
NCH = NT // CHT   # 8 chunks
CHR = CHT * RT + 2  # 18 padded rows per P chunk window
SCH = 16          # rows per xres / phase-C / sumsq2 chunk
NHW = N_IMG * H * W
EPS = 1e-5


def _build_bass(n_cores=N_CORES, nhw=NHW, phase=""):
    nc = bacc.Bacc(
        "TRN2", target_bir_lowering=False, debug=False, num_devices=n_cores
    )

    xP = nc.dram_tensor("xP", [4, 128, HP, WP], BF16, kind="ExternalInput")
    xQ = nc.dram_tensor("xQ", [4, 128, HP, WP], BF16, kind="ExternalInput")
    xR = nc.dram_tensor("xR", [2, 128, H, W], BF16, kind="ExternalInput")
    w1Pd = nc.dram_tensor("w1Pd", [128, 3, C], BF16, kind="ExternalInput")
    w1Qd = nc.dram_tensor("w1Qd", [128, C], BF16, kind="ExternalInput")
    w1Sd = nc.dram_tensor("w1Sd", [C, C], BF16, kind="ExternalInput")
    w2Ad = nc.dram_tensor("w2Ad", [128, 3, C], BF16, kind="ExternalInput")
    w2Bd = nc.dram_tensor("w2Bd", [128, 3, C], BF16, kind="ExternalInput")
    w2Sd = nc.dram_tensor("w2Sd", [128, 3, C], BF16, kind="ExternalInput")
    w2Td = nc.dram_tensor("w2Td", [C, 9, C], BF16, kind="ExternalInput")
    w2Qd = nc.dram_tensor("w2Qd", [128, C], BF16, kind="ExternalInput")
    prmd = nc.dram_tensor("prmd", [128, 4], F32, kind="ExternalInput")
    out = nc.dram_tensor("out", [4, C, H, W], F32, kind="ExternalOutput")

    rg8 = [list(range(n_cores))]

    def _emit(tc, ctx):
        const = ctx.enter_context(tc.tile_pool(name="const", bufs=1))
        panels = ctx.enter_context(tc.tile_pool(name="panels", bufs=1))
        xpp = ctx.enter_context(tc.tile_pool(name="xpp", bufs=4))
        xqp = ctx.enter_context(tc.tile_pool(name="xqp", bufs=3))
        xrp = ctx.enter_context(tc.tile_pool(name="xrp", bufs=4))
        sqp = ctx.enter_context(tc.tile_pool(name="sqp", bufs=2))
        stat = ctx.enter_context(tc.tile_pool(name="stat", bufs=1))
        stg = ctx.enter_context(tc.tile_pool(name="stg", bufs=4))
        qp = ctx.enter_context(tc.tile_pool(name="qp", bufs=3))
        psum = ctx.enter_context(tc.tile_pool(name="psum", bufs=6, space="PSUM"))
        psb = ctx.enter_context(tc.tile_pool(name="psb", bufs=1, space="PSUM"))
        dram = ctx.enter_context(tc.tile_pool(name="dram", bufs=4, space="DRAM"))

        # ---- weights / params ----
        w1P = const.tile([128, 3, C], BF16, tag="w1P")
        w1Q = const.tile([128, C], BF16, tag="w1Q")
        w1S = const.tile([C, C], BF16, tag="w1S")
        w2A = const.tile([128, 3, C], BF16, tag="w2A")
        w2B = const.tile([128, 3, C], BF16, tag="w2B")
        w2S = const.tile([128, 3, C], BF16, tag="w2S")
        w2T = const.tile([C, 9, C], BF16, tag="w2T")
        w2Q = const.tile([128, C], BF16, tag="w2Q")
        prm = const.tile([128, 4], F32, tag="prm")
        for sb, d in ((w1P, w1Pd), (w1Q, w1Qd), (w1S, w1Sd), (w2A, w2Ad),
                      (w2B, w2Bd), (w2S, w2Sd), (w2T, w2Td), (w2Q, w2Qd),
                      (prm, prmd)):
            nc.scalar.dma_start(out=sb[:], in_=d[:])
        ones = const.tile([C, 1, W], BF16, tag="ones")
        nc.gpsimd.memset(ones[:], 1.0)

        # scaled conv2 weights (built after cc1)
        w2sA = const.tile([128, 3, C], BF16, tag="w2sA")
        w2sB = const.tile([128, 3, C], BF16, tag="w2sB")
        w2sS = const.tile([128, 3, C], BF16, tag="w2sS")
        w2sQ = const.tile([128, C], BF16, tag="w2sQ")

        # ---- persistent panels (conv1 out -> conv2 in -> o3) ----
        # pan[img]: [plain-img | shift(1,0)-img] for even img (A-layout),
        #           [shift(1,0)-img | plain-img] for odd img (B-layout).
        pan = [
            panels.tile([128, HP, WP], BF16, tag=f"pan{i}", name=f"pan{i}")
            for i in range(4)
        ]
        for p in pan:
            nc.gpsimd.memset(p[:, 0:1, :], 0.0)
            nc.gpsimd.memset(p[:, HP - 1 : HP, :], 0.0)
            nc.gpsimd.memset(p[:, :, 0:1], 0.0)
            nc.gpsimd.memset(p[:, :, WP - 1 : WP], 0.0)

        # ---- stats ----
        st1 = stat.tile([128, 2 * NT], F32, tag="st1")
        st1q = stat.tile([128, 2 * NT], F32, tag="st1q")
        st2 = stat.tile([128, 2 * NT], F32, tag="st2")
        st2q = stat.tile([128, 2 * H // SCH], F32, tag="st2q")
        red1 = stat.tile([128, 2], F32, tag="red1")
        red2 = stat.tile([128, 2], F32, tag="red2")

        # bias-field / coeff tiles
        fixT = stat.tile([128, 1, W], F32, tag="fixT")
        fixB = stat.tile([128, 1, W], F32, tag="fixB")
        sc = stat.tile([128, 8], F32, tag="sc")
        # sc cols: 0=C, 1=colL, 2=colR, 3=corr2, 4=s2, 5=b2
        m1 = stat.tile([128, 8], F32, tag="m1")
        m2 = stat.tile([128, 8], F32, tag="m2")
        tv = stat.tile([C, 3, 3], F32, tag="tv")
        s1f = stat.tile([128, 1], F32, tag="s1f")
        b1c = stat.tile([C, 1], BF16, tag="b1c")

        def bn_coeffs(tot, gcol, bcol, m, s_out, b_out):
            """tot [128,2] (sum, sumsq) -> scale/bias [128,1] f32 (dual-half)."""
            nc.vector.tensor_scalar(m[:, 0:2], tot[:, 0:2], 1.0 / nhw, None, MULT)
            nc.vector.tensor_tensor(m[:, 2:3], m[:, 0:1], m[:, 0:1], MULT)
            nc.vector.tensor_scalar(
                m[:, 3:4], m[:, 1:2], m[:, 2:3], EPS, SUB, op1=ADD
            )  # var + eps
            nc.vector.reciprocal(m[:, 6:7], m[:, 3:4])
            nc.scalar.activation(m[:, 4:5], m[:, 6:7], AF.Sqrt)
            nc.vector.tensor_tensor(
                s_out, prm[:, gcol : gcol + 1], m[:, 4:5], MULT
            )
            nc.vector.tensor_tensor(m[:, 5:6], m[:, 0:1], s_out, MULT)
            nc.vector.tensor_tensor(
                b_out, prm[:, bcol : bcol + 1], m[:, 5:6], SUB
            )

        def do_collective(src, cc_name):
            # AllGather the raw per-core [128,2] (sum, sumsq) stats, then
            # reduce over (core, partition-half) on-chip. An AllGather of
            # this size is ~2x cheaper than an AllReduce and needs no
            # pre-fold DMAs.
            cc_in = dram.tile([128, 2], F32, tag=cc_name + "i")
            cc_out = dram.tile([n_cores, 2, C, 2], F32, tag=cc_name + "o")
            nc.sync.dma_start(out=cc_in[:], in_=src[:])
            if os.environ.get("KERNEL_NOCC"):
                nc.sync.dma_start(out=cc_out[0, 0], in_=cc_in[0:C, :])
                nc.sync.dma_start(out=cc_out[0, 1], in_=cc_in[C:128, :])
                for k in range(1, n_cores):
                    nc.gpsimd.memset(cc_out[k], 0.0)
            else:
                nc.gpsimd.collective_compute(
                    "AllGather", mybir.AluOpType.bypass, replica_groups=rg8,
                    ins=[cc_in[:].opt()], outs=[cc_out[:].opt()],
                )
            g = stat.tile([128, 2, 2 * n_cores], F32, tag=cc_name + "g")
            gv = cc_out[:].rearrange("k h c s -> c s (k h)")
            nc.sync.dma_start(out=g[0:C], in_=gv)
            nc.scalar.dma_start(out=g[C:128], in_=gv)
            tot = stat.tile([128, 2], F32, tag=cc_name + "t")
            nc.vector.tensor_reduce(tot[:], g[:], AX, ADD)
            return tot

        # ================= Phase A: conv1 =================
        for pair in (0, 1):
            pA, pB = pan[2 * pair], pan[2 * pair + 1]
            for ch in range(NCH):
                r0 = CHT * RT * ch
                cps, cqs = [], []
                for i01 in (0, 1):
                    img = 2 * pair + i01
                    cp = xpp.tile([128, CHR, WP], BF16, tag="xp")
                    cq = xqp.tile([128, CHR - 2, WP], BF16, tag="xq")
                    if pair == 0 and ch == 0:
                        # split first loads so tile 0's matmuls start sooner
                        nc.sync.dma_start(
                            out=cp[:, 0:10, :], in_=xP[img][:, 0:10, :]
                        )
                        nc.sync.dma_start(
                            out=cq[:, 0:8, :], in_=xQ[img][:, 2:10, :]
                        )
                        nc.sync.dma_start(
                            out=cp[:, 10:CHR, :], in_=xP[img][:, 10:CHR, :]
                        )
                        nc.sync.dma_start(
                            out=cq[:, 8 : CHR - 2, :],
                            in_=xQ[img][:, 10 : r0 + CHR, :],
                        )
                    else:
                        nc.sync.dma_start(
                            out=cp[:], in_=xP[img][:, r0 : r0 + CHR, :]
                        )
                        nc.sync.dma_start(
                            out=cq[:], in_=xQ[img][:, r0 + 2 : r0 + CHR, :]
                        )
                    cps.append(cp)
                    cqs.append(cq)
                for tl in range(CHT):
                    t = CHT * ch + tl
                    si = NT * pair + t
                    l0 = RT * tl
                    ps = psum.tile([128, RT, W], F32, tag="ps")
                    for i01 in (0, 1):
                        tp = (0, 64 * i01)
                        po = ps[64 * i01 : 64 * i01 + 64, :, :]
                        cp, cq = cps[i01], cqs[i01]
                        for kx in range(3):
                            nc.tensor.matmul(
                                po, w1P[:, kx, :],
                                cp[:, l0 : l0 + RT, kx : kx + W],
                                start=(kx == 0), stop=False, tile_position=tp,
                            )
                        nc.tensor.matmul(
                            po, w1Q[:, :], cq[:, l0 : l0 + RT, 0:W],
                            start=False, stop=False, tile_position=tp,
                        )
                        nc.tensor.matmul(
                            po, w1S[:, :],
                            cp[0:C, l0 + 2 : l0 + 2 + RT, 2 : 2 + W],
                            start=False, stop=True, tile_position=tp,
                        )
                    # evictions + stats (DVE copies, ACT squares from psum)
                    nc.vector.tensor_scalar(
                        pA[0:C, 1 + RT * t : 1 + RT * t + RT, 1 : 1 + W],
                        ps[0:C], 1.0, 0.0, MULT, op1=ADD,
                        accum_out=st1[0:C, si : si + 1],
                    )
                    nc.vector.tensor_scalar(
                        pB[C:128, 1 + RT * t : 1 + RT * t + RT, 1 : 1 + W],
                        ps[C:128], 1.0, 0.0, MULT, op1=ADD,
                        accum_out=st1[C:128, si : si + 1],
                    )
                    sq = sqp.tile([128, RT, W], BF16, tag="sq")
                    nc.scalar.activation(
                        sq[0:C], ps[0:C], AF.Square,
                        accum_out=st1q[0:C, si : si + 1],
                    )
                    nc.scalar.activation(
                        sq[C:128], ps[C:128], AF.Square,
                        accum_out=st1q[C:128, si : si + 1],
                    )
                # shift copies for this chunk's rows
                rr = CHT * RT * ch
                nr = CHT * RT if ch < NCH - 1 else CHT * RT + 1
                nc.gpsimd.dma_start(
                    out=pA[C:128, rr : rr + nr, :],
                    in_=pA[0:C, rr + 1 : rr + 1 + nr, :],
                )
                nc.gpsimd.dma_start(
                    out=pB[0:C, rr : rr + nr, :],
                    in_=pB[C:128, rr + 1 : rr + 1 + nr, :],
                )

        def emit_raw_out():
            SCC = 8
            for pair in (0, 1):
                pA_, pB_ = pan[2 * pair], pan[2 * pair + 1]
                for ch in range(H // SCC):
                    pr0 = 1 + SCC * ch
                    so = stg.tile([128, SCC, W], F32, tag="so", name="so")
                    nc.scalar.activation(
                        so[0:C, :, :],
                        pA_[0:C, pr0 : pr0 + SCC, 1 : 1 + W], AF.Copy,
                    )
                    nc.vector.tensor_scalar(
                        so[C:128, :, :],
                        pB_[C:128, pr0 : pr0 + SCC, 1 : 1 + W],
                        1.0, None, MULT,
                    )
                    nc.sync.dma_start(
                        out=out[2 * pair, :, SCC * ch : SCC * ch + SCC, :],
                        in_=so[0:C, :, :],
                    )
                    nc.gpsimd.dma_start(
                        out=out[2 * pair + 1, :, SCC * ch : SCC * ch + SCC, :],
                        in_=so[C:128, :, :],
                    )

        if phase == "A":
            emit_raw_out()

        # ================= BN1 stats + fold into w2 =================
        if phase == "A":
            return
        nc.vector.tensor_reduce(red1[:, 0:1], st1[:], AX, ADD)
        nc.vector.tensor_reduce(red1[:, 1:2], st1q[:], AX, ADD)
        tot1 = do_collective(red1, "cc1")
        bn_coeffs(tot1, 0, 1, m1, s1f[:, 0:1], m1[:, 6:7])  # s1, b1
        nc.vector.tensor_scalar(b1c[:], m1[0:C, 6:7], 1.0, None, MULT)
        nc.vector.tensor_scalar(w2sA[:], w2A[:], s1f[:, 0:1], None, MULT)
        nc.vector.tensor_scalar(w2sB[:], w2B[:], s1f[:, 0:1], None, MULT)
        nc.vector.tensor_scalar(w2sS[:], w2S[:], s1f[:, 0:1], None, MULT)
        nc.vector.tensor_scalar(w2sQ[:], w2Q[:], s1f[:, 0:1], None, MULT)

        # bias field: tv[o, ky, kx] = sum_i w2[o,i,ky,kx] * b1[i]
        pstv = psb.tile([C, 16], F32, tag="pstv")
        for tap in range(9):
            nc.tensor.matmul(
                pstv[:, tap : tap + 1], w2T[:, tap, :], b1c[:, 0:1],
                start=True, stop=True, tile_position=(0, 0),
            )
        nc.scalar.activation(tv[:, :, :], pstv[:, 0:9], AF.Copy)
        # class sums (all [C,1]):
        nc.vector.tensor_reduce(m2[0:C, 0:1], tv[:, :, :], AXY, ADD)  # Csum
        nc.vector.tensor_reduce(m2[0:C, 1:2], tv[:, 0:1, :], AXY, ADD)  # dT
        nc.vector.tensor_reduce(m2[0:C, 2:3], tv[:, 2:3, :], AXY, ADD)  # dB
        nc.vector.tensor_reduce(m2[0:C, 3:4], tv[:, :, 0:1], AXY, ADD)  # dL
        nc.vector.tensor_reduce(m2[0:C, 4:5], tv[:, :, 2:3], AXY, ADD)  # dR
        # fix rows: fixT = -dT everywhere; corners -dT-dL+T00 / -dT-dR+T02
        nc.vector.tensor_scalar(
            fixT[0:C, :, :], ones[:], m2[0:C, 1:2], -1.0, MULT, op1=MULT
        )
        nc.vector.tensor_scalar(
            fixB[0:C, :, :], ones[:], m2[0:C, 2:3], -1.0, MULT, op1=MULT
        )
        # corner deltas: m2[0:C,5] = T00 - dL etc; then add -dT
        nc.vector.tensor_tensor(m2[0:C, 5:6], tv[:, 0, 0:1], m2[0:C, 3:4], SUB)
        nc.vector.tensor_tensor(
            fixT[0:C, 0, 0:1], m2[0:C, 5:6], m2[0:C, 1:2], SUB
        )
        nc.vector.tensor_tensor(m2[0:C, 5:6], tv[:, 0, 2:3], m2[0:C, 4:5], SUB)
        nc.vector.tensor_tensor(
            fixT[0:C, 0, W - 1 : W], m2[0:C, 5:6], m2[0:C, 1:2], SUB
        )
        nc.vector.tensor_tensor(m2[0:C, 5:6], tv[:, 2, 0:1], m2[0:C, 3:4], SUB)
        nc.vector.tensor_tensor(
            fixB[0:C, 0, 0:1], m2[0:C, 5:6], m2[0:C, 2:3], SUB
        )
        nc.vector.tensor_tensor(m2[0:C, 5:6], tv[:, 2, 2:3], m2[0:C, 4:5], SUB)
        nc.vector.tensor_tensor(
            fixB[0:C, 0, W - 1 : W], m2[0:C, 5:6], m2[0:C, 2:3], SUB
        )
        # sc: C, colL=-dL, colR=-dR, corr2
        nc.vector.tensor_scalar(sc[0:C, 0:1], m2[0:C, 0:1], 1.0, None, MULT)
        nc.vector.tensor_scalar(sc[0:C, 1:2], m2[0:C, 3:4], -1.0, None, MULT)
        nc.vector.tensor_scalar(sc[0:C, 2:3], m2[0:C, 4:5], -1.0, None, MULT)
        # corr2 = 2*( sum(fixT) + sum(fixB) + (H-2)*(colL+colR) )
        nc.vector.tensor_reduce(m2[0:C, 5:6], fixT[0:C, :, :], AXY, ADD)
        nc.vector.tensor_reduce(m2[0:C, 6:7], fixB[0:C, :, :], AXY, ADD)
        nc.vector.tensor_tensor(m2[0:C, 7:8], sc[0:C, 1:2], sc[0:C, 2:3], ADD)
        nc.vector.tensor_scalar(m2[0:C, 7:8], m2[0:C, 7:8], float(H - 2), None, MULT)
        nc.vector.tensor_tensor(m2[0:C, 5:6], m2[0:C, 5:6], m2[0:C, 6:7], ADD)
        nc.vector.tensor_tensor(m2[0:C, 5:6], m2[0:C, 5:6], m2[0:C, 7:8], ADD)
        nc.vector.tensor_scalar(sc[0:C, 3:4], m2[0:C, 5:6], 2.0, None, MULT)
        # duplicate to upper half
        nc.sync.dma_start(out=fixT[C:128, :, :], in_=fixT[0:C, :, :])
        nc.sync.dma_start(out=fixB[C:128, :, :], in_=fixB[0:C, :, :])
        nc.sync.dma_start(out=sc[C:128, 0:4], in_=sc[0:C, 0:4])

        # ================= Phase B: conv2 + residual =================
        # Evictions lag the matmuls by one tile: evict(t-1) overwrites padded
        # row 4t (o1 -> o3 in place), which MM(t) still reads as o1. Emitting
        # MM(t) first makes the framework order the overwrite after the read.
        for pair in (0, 1):
            pA, pB = pan[2 * pair], pan[2 * pair + 1]

            def evict2(t, ps, xr, lr, pA=pA, pB=pB, pair=pair):
                si = NT * pair + t
                tmp = sqp.tile([128, RT, W], BF16, tag="sq", name="tmp")
                nc.vector.tensor_tensor(
                    tmp[:], ps[:], xr[:, lr : lr + RT, :], ADD
                )
                nc.vector.tensor_scalar(
                    pA[0:C, 1 + RT * t : 1 + RT * t + RT, 1 : 1 + W],
                    tmp[0:C], sc[0:C, 0:1], 0.0, SUB, op1=ADD,
                    accum_out=st2[0:C, si : si + 1],
                )
                nc.vector.tensor_scalar(
                    pB[C:128, 1 + RT * t : 1 + RT * t + RT, 1 : 1 + W],
                    tmp[C:128], sc[C:128, 0:1], 0.0, SUB, op1=ADD,
                    accum_out=st2[C:128, si : si + 1],
                )

            def fix2(ch, pA=pA, pB=pB, pair=pair):
                pr0 = 1 + SCH * ch
                for pp, lo, hi in ((pA, 0, C), (pB, C, 128)):
                    if ch == 0:
                        nc.vector.tensor_tensor(
                            pp[lo:hi, 1:2, 1 : 1 + W],
                            pp[lo:hi, 1:2, 1 : 1 + W],
                            fixT[lo:hi, :, :], ADD,
                        )
                    if ch == H // SCH - 1:
                        nc.vector.tensor_tensor(
                            pp[lo:hi, HP - 2 : HP - 1, 1 : 1 + W],
                            pp[lo:hi, HP - 2 : HP - 1, 1 : 1 + W],
                            fixB[lo:hi, :, :], ADD,
                        )
                    ra = pr0 + 1 if ch == 0 else pr0
                    rb = pr0 + SCH - 1 if ch == H // SCH - 1 else pr0 + SCH
                    nc.vector.tensor_scalar(
                        pp[lo:hi, ra:rb, 1:2],
                        pp[lo:hi, ra:rb, 1:2],
                        sc[lo:hi, 1:2], None, ADD,
                    )
                    nc.vector.tensor_scalar(
                        pp[lo:hi, ra:rb, W : W + 1],
                        pp[lo:hi, ra:rb, W : W + 1],
                        sc[lo:hi, 2:3], None, ADD,
                    )
                qi = (H // SCH) * pair + ch
                sq2 = stg.tile([128, SCH, W], BF16, tag="so", name="sq2")
                nc.scalar.activation(
                    sq2[0:C, :, :],
                    pA[0:C, pr0 : pr0 + SCH, 1 : 1 + W],
                    AF.Square,
                    accum_out=st2q[0:C, qi : qi + 1],
                )
                nc.scalar.activation(
                    sq2[C:128, :, :],
                    pB[C:128, pr0 : pr0 + SCH, 1 : 1 + W],
                    AF.Square,
                    accum_out=st2q[C:128, qi : qi + 1],
                )

            pend = None
            for ch in range(H // SCH):
                xrs = []
                for hh in (0, 1):
                    xr_ = xrp.tile([128, SCH // 2, W], BF16, tag="xr", name="xr_")
                    r0x = SCH * ch + (SCH // 2) * hh
                    nc.sync.dma_start(
                        out=xr_[:], in_=xR[pair][:, r0x : r0x + SCH // 2, :]
                    )
                    xrs.append(xr_)
                # Q chunks: [o1 col-shifted | o1 plain], copied from the
                # panels' plain halves before the in-place o3 overwrite
                qr0 = SCH * ch + 2
                qts = []
                for i01 in (0, 1):
                    qt = qp.tile([128, SCH, WP], BF16, tag="q", name="qt")
                    pp = pA if i01 == 0 else pB
                    src_half = pp[0:C] if i01 == 0 else pp[C:128]
                    nc.sync.dma_start(
                        out=qt[0:C, :, 0 : WP - 1],
                        in_=src_half[:, qr0 : qr0 + SCH, 1:WP],
                    )
                    nc.gpsimd.dma_start(
                        out=qt[C:128],
                        in_=src_half[:, qr0 : qr0 + SCH, :],
                    )
                    qts.append(qt)
                for tl in range(SCH // RT):
                    t = (SCH // RT) * ch + tl
                    lr = RT * tl
                    ps = psum.tile([128, RT, W], F32, tag="ps", name="ps")
                    for i01 in (0, 1):
                        tp = (0, 64 * i01)
                        po = ps[64 * i01 : 64 * i01 + 64, :, :]
                        pp = pA if i01 == 0 else pB
                        wP = w2sA if i01 == 0 else w2sB
                        for kx in range(3):
                            nc.tensor.matmul(
                                po, wP[:, kx, :],
                                pp[:, RT * t : RT * t + RT, kx : kx + W],
                                start=(kx == 0), stop=False, tile_position=tp,
                            )
                        nc.tensor.matmul(
                            po, w2sQ[:, :],
                            qts[i01][:, lr : lr + RT, 0:W],
                            start=False, stop=False, tile_position=tp,
                        )
                        nc.tensor.matmul(
                            po,
                            w2sS[64 * i01 : 64 * i01 + 64, 2, :],
                            pp[
                                64 * i01 : 64 * i01 + 64,
                                RT * t + 2 : RT * t + 2 + RT,
                                2 : 2 + W,
                            ],
                            start=False, stop=True,
                            tile_position=(64 * i01, 64 * i01),
                        )
                    if pend is not None:
                        evict2(*pend)
                    pend = (t, ps, xrs[tl // 2], RT * (tl % 2))
                if ch > 0:
                    fix2(ch - 1)
            evict2(*pend)
            fix2(H // SCH - 1)

        if phase == "B":
            emit_raw_out()

        # ================= BN2 stats =================
        if phase == "B":
            return
        nc.vector.tensor_reduce(red2[:, 0:1], st2[:], AX, ADD)
        nc.vector.tensor_scalar(
            red2[:, 0:1], red2[:, 0:1], sc[:, 3:4], None, ADD
        )
        nc.vector.tensor_reduce(red2[:, 1:2], st2q[:], AX, ADD)
        tot2 = do_collective(red2, "cc2")
        bn_coeffs(tot2, 2, 3, m2, sc[:, 4:5], sc[:, 5:6])  # s2, b2

        # ================= Phase C: BN2 apply + store =================
        SCC = 8
        for ch in range(H // SCC):
            for pair in (0, 1):
                pA, pB = pan[2 * pair], pan[2 * pair + 1]
                pr0 = 1 + SCC * ch
                so = stg.tile([128, SCC, W], F32, tag="so")
                nc.scalar.activation(
                    so[0:C, :, :],
                    pA[0:C, pr0 : pr0 + SCC, 1 : 1 + W],
                    AF.Identity,
                    bias=sc[0:C, 5:6], scale=sc[0:C, 4:5],
                )
                nc.vector.tensor_scalar(
                    so[C:128, :, :],
                    pB[C:128, pr0 : pr0 + SCC, 1 : 1 + W],
                    sc[C:128, 4:5], sc[C:128, 5:6], MULT, op1=ADD,
                )
                nc.sync.dma_start(
                    out=out[2 * pair, :, SCC * ch : SCC * ch + SCC, :],
                    in_=so[0:C, :, :],
                )
                nc.gpsimd.dma_start(
                    out=out[2 * pair + 1, :, SCC * ch : SCC * ch + SCC, :],
                    in_=so[C:128, :, :],
                )

    with tile.TileContext(nc) as tc, ExitStack() as ctx:
        _emit(tc, ctx)
    nc.finalize()
    return nc


_NC_CACHE = {}


def _prep_inputs(inputs):
    x = np.asarray(inputs["x"], dtype=np.float32)
    w1 = np.asarray(inputs["w1"], dtype=np.float32)
    w2 = np.asarray(inputs["w2"], dtype=np.float32)
    g1 = np.asarray(inputs["bn1_gamma"], dtype=np.float32)
    b1 = np.asarray(inputs["bn1_beta"], dtype=np.float32)
    g2 = np.asarray(inputs["bn2_gamma"], dtype=np.float32)
    b2 = np.asarray(inputs["bn2_beta"], dtype=np.float32)
    bf = ml_dtypes.bfloat16

    xpad = np.zeros((N_IMG, C, HP, WP), np.float32)
    xpad[:, :, 1 : 1 + H, 1 : 1 + W] = x
    sh10 = np.zeros_like(xpad)
    sh10[:, :, 0 : HP - 1, :] = xpad[:, :, 1:HP, :]
    sh01 = np.zeros_like(xpad)
    sh01[:, :, :, 0 : WP - 1] = xpad[:, :, :, 1:WP]

    xP = np.concatenate([xpad, sh10], axis=1).astype(bf)   # [32, 128, HP, WP]
    xQ = np.concatenate([sh01, xpad], axis=1).astype(bf)
    xRf = x.reshape(N_IMG // 2, 2 * C, H, W).astype(bf)     # [16, 128, H, W]

    w1t = np.ascontiguousarray(w1.transpose(1, 2, 3, 0))   # [i, ky, kx, o]
    w2t = np.ascontiguousarray(w2.transpose(1, 2, 3, 0))
    w1P = np.concatenate([w1t[:, 0], w1t[:, 1]], axis=0).astype(bf)
    w1Q = np.concatenate([w1t[:, 2, 1], w1t[:, 2, 0]], axis=0).astype(bf)
    w1S = np.ascontiguousarray(w1t[:, 2, 2]).astype(bf)
    w2A = np.concatenate([w2t[:, 0], w2t[:, 1]], axis=0).astype(bf)
    w2B = np.concatenate([w2t[:, 1], w2t[:, 0]], axis=0).astype(bf)
    w2Sv = np.concatenate([w2t[:, 2], w2t[:, 2]], axis=0).astype(bf)
    w2T = np.ascontiguousarray(w2t.reshape(C, 9, C)).astype(bf)
    w2Qv = np.concatenate([w2t[:, 2, 1], w2t[:, 2, 0]], axis=0).astype(bf)
    prmv = np.tile(np.stack([g1, b1, g2, b2], axis=1), (2, 1)).astype(np.float32)

    in_maps = []
    for k in range(N_CORES):
        in_maps.append({
            "xP": np.ascontiguousarray(xP[4 * k : 4 * k + 4]),
            "xQ": np.ascontiguousarray(xQ[4 * k : 4 * k + 4]),
            "xR": np.ascontiguousarray(xRf[2 * k : 2 * k + 2]),
            "w1Pd": w1P, "w1Qd": w1Q, "w1Sd": w1S,
            "w2Ad": w2A, "w2Bd": w2B, "w2Sd": w2Sv, "w2Td": w2T,
            "w2Qd": w2Qv,
            "prmd": prmv,
        })
    return in_maps


def kernel(**inputs):
    ph = os.environ.get("KERNEL_PH", "")
    if ph not in _NC_CACHE:
        _NC_CACHE[ph] = _build_bass(phase=ph)
    nc = _NC_CACHE[ph]
    in_maps = _prep_inputs(inputs)
    trace = bool(int(os.environ.get("KERNEL_TRACE", "0")))
    res = run_bass_kernel_spmd(
        nc, in_maps, core_ids=list(range(N_CORES)), trace=trace
    )
    if trace:
        kernel.last_exec_time_ns = res.exec_time_ns
        kernel.last_results = res
    out = np.concatenate([r["out"] for r in res.results], axis=0)
    return out.astype(np.float32)


if __name__ == "__main__":
    nc = _build_bass()
    print("build ok")
